# revision 20
# baseline (speedup 1.0000x reference)
"""GAT+GCN Trainium2 kernel: 8-core SPMD Bass/Tile implementation.

Sharding: nodes/graphs split contiguously across cores.  Edges (without
self loops) assigned to the core owning their dst node, sorted by dst,
padded per 128-node dst tile to K chunks of 128 edges; self-loop terms
are applied analytically (identity selection / per-partition scalars).

GAT restructure: by linearity, messages aggregate in x-space first
(aggx[d,h,:] = sum_e w_e x[src_e]) and the per-head gat_w transform is
applied after aggregation.  The AllGathered table is therefore only
[x | 1 | a_src] = 89 fp8 bytes/row instead of 790 (h | a_src).  The
constant-1 column aggregates to the softmax denominator, which both
normalizes and (with a bias row appended to gat_w) folds in gat_b
exactly.  a_src/a_dst projections of x are folded on the host, so the
first AllGather starts at t=0.  The second table (g1) is fp8.  Dense
matmuls run in fp16; MLP weights prefetch to SBUF during the first
AllGather; the protein-branch conv runs in the AllGather shadows.
"""
import numpy as np
import ml_dtypes
import concourse.bass as bass
import concourse.bacc as bacc
import concourse.mybir as mybir
import concourse.tile as tile

f32 = mybir.dt.float32
f16 = mybir.dt.float16
f8 = mybir.dt.float8e4
i32 = mybir.dt.int32
AF = mybir.ActivationFunctionType
OP = mybir.AluOpType
AX = mybir.AxisListType

F = 78          # input feature dim
H = 10          # heads
HID = 780       # F*H
XF = F + 1      # x | 1
XW = XF + H     # xtab row: x | 1 | a_src
AGW = (XF) * H  # aggregated width (h, x|denom) = 790
NKK = 7         # ceil(HID/128)


def ceil_div(a, b):
    return (a + b - 1) // b


def _chunk_rows(w, n_chunks, rows_per=128):
    """Pack w[rows, cols] row-chunks into [128, n_chunks*cols] lhsT tiles."""
    rows, cols = w.shape
    out = np.zeros((n_chunks, rows_per, cols), np.float32)
    for j in range(n_chunks):
        ks = j * rows_per
        kn = min(rows_per, rows - ks)
        if kn > 0:
            out[j, :kn] = w[ks:ks + kn]
    return np.ascontiguousarray(
        out.transpose(1, 0, 2).reshape(rows_per, n_chunks * cols))


def host_prep(inp, n_cores=8):
    """Build per-core input maps + cfg from full inputs."""
    x = np.asarray(inp["x"], np.float32)
    ei = np.asarray(inp["edge_index"], np.int64)
    tgt = np.asarray(inp["target"], np.int64)
    N = x.shape[0]
    B = tgt.shape[0]
    GN = N // B                # nodes per graph
    NS = N // n_cores
    T = NS // 128
    BL = B // n_cores

    src = ei[0]
    dst = ei[1]
    E = src.shape[0]

    # degrees include the self loops the reference adds
    deg = np.bincount(dst, minlength=N).astype(np.float64) + 1.0
    dinv = 1.0 / np.sqrt(deg)
    normv = (dinv[src] * dinv[dst]).astype(np.float32)
    nself = (dinv * dinv).astype(np.float32)

    order = np.argsort(dst, kind="stable")
    src_s = src[order].astype(np.int32)
    dst_s = dst[order].astype(np.int32)
    norm_s = normv[order]

    gtile = dst_s // 128
    n_gtiles = N // 128
    starts = np.searchsorted(gtile, np.arange(n_gtiles))
    cnts = np.searchsorted(gtile, np.arange(n_gtiles), side="right") - starts
    K = int(np.max(ceil_div(cnts, 128)))

    srcs_p = np.zeros((n_gtiles, 128, K), np.int32)
    dstf_p = np.full((n_gtiles, 128, K), 128.0, np.float32)
    norm_p = np.zeros((n_gtiles, 128, K), np.float32)
    j = np.arange(E) - starts[gtile]
    srcs_p[gtile, j % 128, j // 128] = src_s
    dstf_p[gtile, j % 128, j // 128] = (dst_s % 128).astype(np.float32)
    norm_p[gtile, j % 128, j // 128] = norm_s

    gat_w = np.asarray(inp["gat_w"], np.float32)
    gat_b = np.asarray(inp["gat_b"], np.float32)
    att_src = np.asarray(inp["att_src"], np.float32)
    att_dst = np.asarray(inp["att_dst"], np.float32)
    As = np.einsum("fhc,hc->fh", gat_w.reshape(F, H, F), att_src)
    Ad = np.einsum("fhc,hc->fh", gat_w.reshape(F, H, F), att_dst)

    # xtab row: [x (78) | 1 | a_src (10)] in fp8
    a_src_n = x @ As
    a_dst_n = (x @ Ad).astype(np.float16)
    # per-edge-slot a_dst values (host gather by dst)
    adste_p = np.zeros((n_gtiles, 128, K, H), np.float16)
    adste_p[gtile, j % 128, j // 128] = a_dst_n[dst_s]
    xt = np.zeros((N, XW), np.float32)
    xt[:, :F] = x
    xt[:, F] = 1.0
    xt[:, XF:] = a_src_n
    xtab_np = xt.astype(ml_dtypes.float8_e4m3)

    # per-head transform [x|denom] -> h: rows 0..77 = gat_w head block,
    # row 78 = gat_b head block (the denom column normalizes to 1).
    whead = np.zeros((128, HID), np.float32)
    for h in range(H):
        whead[:F, h * F:(h + 1) * F] = gat_w[:, h * F:(h + 1) * F]
        whead[F, h * F:(h + 1) * F] = gat_b[h * F:(h + 1) * F]
    whead16 = whead.astype(np.float16)

    gcn_w = np.asarray(inp["gcn_w"], np.float32)
    gcn_b = np.asarray(inp["gcn_b"], np.float32)
    # bias folded as an extra contraction row (s16 carries a 1.0 column)
    gcn_ext = np.concatenate([gcn_w, gcn_b[None, :]], 0)   # [781, 780]
    gcnw16 = _chunk_rows(gcn_ext, NKK).astype(np.float16)

    fcg1_w = np.asarray(inp["fcg1_w"], np.float32).copy()
    fcg1_w[HID:] *= 1.0 / GN
    f14 = np.zeros((14, 128, 1500), np.float32)
    for jj in range(NKK):
        ks = jj * 128
        kn = min(128, HID - ks)
        f14[jj, :kn] = fcg1_w[ks:ks + kn]
        f14[NKK + jj, :kn] = fcg1_w[HID + ks:HID + ks + kn]
    fcg1w16 = np.ascontiguousarray(
        f14.transpose(1, 0, 2).reshape(128, 14 * 1500)).astype(np.float16)

    def bias_sw(b, mt):
        b = np.asarray(b, np.float32)
        out = np.zeros((mt * 128,), np.float32)
        out[: b.shape[0]] = b
        return np.ascontiguousarray(out.reshape(mt, 128).T)

    fcg1_bsw = bias_sw(inp["fcg1_b"], 12)
    fcg2_w = np.asarray(inp["fcg2_w"], np.float32)
    fcg2w16 = _chunk_rows(fcg2_w, 12).astype(np.float16)
    fcg2_bsw = bias_sw(inp["fcg2_b"], 1)

    convxt_w = np.asarray(inp["convxt_w"], np.float32)
    W2 = np.ascontiguousarray(convxt_w.transpose(1, 2, 0).reshape(1000, 8 * 32))
    w2_16 = _chunk_rows(W2, 8).astype(np.float16)
    emb16 = np.asarray(inp["emb"], np.float32).astype(np.float16)
    fcxt_w = np.asarray(inp["fcxt_w"], np.float32)
    fxp = np.zeros((128, 32 * 128), np.float32)
    fxp[:121] = fcxt_w.reshape(32, 121, 128).transpose(1, 0, 2).reshape(
        121, 32 * 128)
    fcxtw16 = fxp.astype(np.float16)
    cb = np.asarray(inp["convxt_b"], np.float32)
    bias_fold = (cb[:, None] * fcxt_w.reshape(32, 121, 128).sum(1)).sum(0)
    fcxt_bsw = bias_sw(np.asarray(inp["fcxt_b"], np.float32) + bias_fold, 1)

    fc1_w = np.asarray(inp["fc1_w"], np.float32)
    fc1w16 = _chunk_rows(fc1_w, 2).astype(np.float16)
    fc1_bsw = bias_sw(inp["fc1_b"], 8)
    fc2_w = np.asarray(inp["fc2_w"], np.float32)
    fc2w16 = _chunk_rows(fc2_w, 8).astype(np.float16)
    fc2_bsw = bias_sw(inp["fc2_b"], 4)
    out_w = np.asarray(inp["out_w"], np.float32)
    outw16 = np.ascontiguousarray(out_w.reshape(4, 128).T).astype(np.float16)
    out_b = np.asarray(inp["out_b"], np.float32).reshape(1, 1)

    iota128 = np.broadcast_to(
        np.arange(128, dtype=np.float16), (128, 128)).copy()
    iota26 = np.broadcast_to(
        np.tile(np.arange(26, dtype=np.float16), 8), (128, 8 * 26)).copy()
    ident16 = np.eye(128, dtype=np.float16)
    ng = 128 // GN
    poolm16 = np.zeros((128, ng), np.float16)
    for g in range(ng):
        poolm16[g * GN:(g + 1) * GN, g] = 1.0

    shared = dict(
        whead=whead16, gcnw=gcnw16, fcg1w=fcg1w16, fcg1_bsw=fcg1_bsw,
        fcg2w=fcg2w16, fcg2_bsw=fcg2_bsw, w2=w2_16, emb=emb16,
        fcxtw=fcxtw16, fcxt_bsw=fcxt_bsw, fc1w=fc1w16, fc1_bsw=fc1_bsw,
        fc2w=fc2w16, fc2_bsw=fc2_bsw, outw=outw16, out_b=out_b,
        iota128=iota128, iota26=iota26, ident16=ident16, poolm=poolm16,
    )

    in_maps = []
    for c in range(n_cores):
        m = dict(shared)
        gt = slice(c * T, (c + 1) * T)
        m["xtab_sl"] = np.ascontiguousarray(xtab_np[c * NS:(c + 1) * NS])
        m["adst"] = np.ascontiguousarray(
            a_dst_n[c * NS:(c + 1) * NS].reshape(T, 128, H)
            .transpose(1, 0, 2).reshape(128, T * H))
        m["srcs"] = np.ascontiguousarray(srcs_p[gt].reshape(T * 128, K))
        m["adste"] = np.ascontiguousarray(
            adste_p[gt].reshape(T * 128, K * H))
        m["dstf"] = np.ascontiguousarray(dstf_p[gt].reshape(T * 128, K))
        m["normv"] = np.ascontiguousarray(norm_p[gt].reshape(T * 128, K))
        m["nself"] = np.ascontiguousarray(
            nself[c * NS:(c + 1) * NS].reshape(T, 128).T)
        tpad = np.zeros((BL, 1024), np.int64)
        tpad[:, :1000] = tgt[c * BL:(c + 1) * BL]
        tl = tpad.reshape(BL, 8, 128)
        m["t_sb"] = np.ascontiguousarray(
            tl.transpose(2, 0, 1).reshape(128, BL * 8).astype(np.float16))
        in_maps.append(m)

    cfg = dict(n_cores=n_cores, N=N, NS=NS, T=T, BL=BL, K=K, GN=GN)
    return in_maps, cfg


def build(cfg, ablate=()):
    n_cores, NS, T, BL, K, GN = (cfg["n_cores"], cfg["NS"], cfg["T"],
                                 cfg["BL"], cfg["K"], cfg["GN"])
    N = cfg["N"]
    KT = K + 1          # +1 chunk for self loops
    ng = 128 // GN

    nc = bacc.Bacc(None, target_bir_lowering=False)

    def dinp(name, shape, dt=f32):
        return nc.dram_tensor(name, list(shape), dt, kind="ExternalInput")

    xtab_d = dinp("xtab_sl", (NS, XW), f8)
    adst_d = dinp("adst", (128, T * H), f16)
    srcs = dinp("srcs", (T * 128, K), i32)
    adste_d = dinp("adste", (T * 128, K * H), f16)
    dstf = dinp("dstf", (T * 128, K))
    normv = dinp("normv", (T * 128, K))
    nself_d = dinp("nself", (128, T))
    t_sb_d = dinp("t_sb", (128, BL * 8), f16)
    whead_d = dinp("whead", (128, HID), f16)
    gcnw_d = dinp("gcnw", (128, NKK * HID), f16)
    fcg1w_d = dinp("fcg1w", (128, 14 * 1500), f16)
    fcg1b_d = dinp("fcg1_bsw", (128, 12))
    fcg2w_d = dinp("fcg2w", (128, 12 * 128), f16)
    fcg2b_d = dinp("fcg2_bsw", (128, 1))
    w2_d = dinp("w2", (128, 8 * 256), f16)
    emb_d = dinp("emb", (26, 128), f16)
    fcxtw_d = dinp("fcxtw", (128, 32 * 128), f16)
    fcxtb_d = dinp("fcxt_bsw", (128, 1))
    fc1w_d = dinp("fc1w", (128, 2 * 1024), f16)
    fc1b_d = dinp("fc1_bsw", (128, 8))
    fc2w_d = dinp("fc2w", (128, 8 * 512), f16)
    fc2b_d = dinp("fc2_bsw", (128, 4))
    outw_d = dinp("outw", (128, 4), f16)
    outb_d = dinp("out_b", (1, 1))
    iota_d = dinp("iota128", (128, 128), f16)
    iota26_d = dinp("iota26", (128, 8 * 26), f16)
    ident_d = dinp("ident16", (128, 128), f16)
    poolm_d = dinp("poolm", (128, ng), f16)

    outp = nc.dram_tensor("outp", [1, BL], f32, kind="ExternalOutput")

    xtab_in = nc.dram_tensor("xtab_in", [NS, XW], f8)
    xtab = nc.dram_tensor("xtab", [N, XW], f8, addr_space="Shared")
    g1_sl = nc.dram_tensor("g1_sl", [NS, HID], f8)
    g1tab = nc.dram_tensor("g1tab", [N, HID], f8, addr_space="Shared")
    gap_dram = nc.dram_tensor("gap_dram", [BL, HID], f16)

    # z-matmul contraction chunks over 781 rows (780 + bias row)
    FCH = [(kk * 128, min(128, HID + 1 - kk * 128)) for kk in range(NKK)]
    # pooling/transpose chunks over 780 cols
    PCH = [(kk * 128, min(128, HID - kk * 128)) for kk in range(NKK)]

    def tiles(n, step=128):
        return [(s, min(step, n - s)) for s in range(0, n, step)]

    # per-head output column segments, split at the 512 PSUM bank boundary
    HSEG = []
    for h in range(H):
        a, b = h * F, (h + 1) * F
        if a < 512 < b:
            HSEG.append((h, a, 512))
            HSEG.append((h, 512, b))
        else:
            HSEG.append((h, a, b))

    with tile.TileContext(nc) as tc:
        with (
            tc.tile_pool(name="const", bufs=1) as cpool,
            tc.tile_pool(name="sb", bufs=2) as pool,
            tc.tile_pool(name="sb3", bufs=3) as pool3,
            tc.tile_pool(name="sm", bufs=6) as spool,
            tc.tile_pool(name="ps", bufs=2, space="PSUM") as psp,
            tc.tile_pool(name="pstr", bufs=3, space="PSUM") as pst,
            tc.tile_pool(name="psm", bufs=1, space="PSUM") as psm,
        ):
            # ---- AllGather 1 launches immediately (xtab is an input) ----
            nc.sync.dma_start(
                out=xtab_in[:].rearrange("(p q) b -> p (q b)", p=128),
                in_=xtab_d[:].rearrange("(p q) b -> p (q b)", p=128))
            if "coll" not in ablate:
                nc.gpsimd.collective_compute(
                    "AllGather", OP.bypass,
                    replica_groups=[list(range(n_cores))],
                    ins=[xtab_in[:]], outs=[xtab[:]],
                )
            else:
                nc.gpsimd.dma_start(out=xtab[:NS, :], in_=xtab_in[:])

            # ---------- resident constants ----------
            def load_sp(name, dram, shape, dt=f32):
                t_ = cpool.tile(list(shape), dt, tag=name, name=name)
                nc.sync.dma_start(out=t_[:], in_=dram[:])
                return t_

            def load_pool(name, dram, shape, dt=f16):
                t_ = cpool.tile(list(shape), dt, tag=name, name=name)
                nc.scalar.dma_start(out=t_[:], in_=dram[:])
                return t_

            ident_sb = load_sp("ident", ident_d, [128, 128], f16)
            iota_sb = load_sp("iota", iota_d, [128, 128], f16)
            adst_sb = load_sp("adst", adst_d, [128, T * H], f16)
            whead_sb = load_sp("whead", whead_d, [128, HID], f16)
            nself_sb = load_sp("nself", nself_d, [128, T])
            iota26_sb = load_sp("iota26", iota26_d, [128, 8 * 26], f16)
            t_sb = load_sp("tsb", t_sb_d, [128, BL * 8], f16)
            w2_sb = load_sp("w2", w2_d, [128, 8 * 256], f16)
            emb_sb = load_sp("emb", emb_d, [26, 128], f16)

            gcnw_sb = load_pool("gcnw", gcnw_d, [128, NKK * HID])
            fcg1w_sb = load_pool("fcg1w", fcg1w_d, [128, 14 * 1500])
            fcg2w_sb = load_pool("fcg2w", fcg2w_d, [128, 12 * 128])
            fcxtw_sb = load_pool("fcxtw", fcxtw_d, [128, 32 * 128])
            fc1w_sb = load_pool("fc1w", fc1w_d, [128, 2 * 1024])
            fc2w_sb = load_pool("fc2w", fc2w_d, [128, 8 * 512])
            outw_sb = load_pool("outw", outw_d, [128, 4])
            poolm_sb = load_pool("poolm", poolm_d, [128, ng])
            fcg1b_sb = load_pool("fcg1b", fcg1b_d, [128, 12], f32)
            fcg2b_sb = load_pool("fcg2b", fcg2b_d, [128, 1], f32)
            fcxtb_sb = load_pool("fcxtb", fcxtb_d, [128, 1], f32)
            fc1b_sb = load_pool("fc1b", fc1b_d, [128, 8], f32)
            fc2b_sb = load_pool("fc2b", fc2b_d, [128, 4], f32)
            outb_sb = load_pool("outb", outb_d, [1, 1], f32)

            cvT_sb = cpool.tile([128, BL * 32], f16, tag="cvT")

            # ---- protein branch conv — overlaps AllGather 1 ----
            def conv_block(b_lo, b_hi):
                for b in range(b_lo, b_hi):
                    oh = pool.tile([128, 8 * 26], f16, tag="oh")
                    nc.vector.tensor_tensor(
                        out=oh[:].rearrange("p (k c) -> p k c", c=26),
                        in0=iota26_sb[:].rearrange("p (k c) -> p k c", c=26),
                        in1=t_sb[:, b * 8:(b + 1) * 8].unsqueeze(2)
                            .to_broadcast([128, 8, 26]),
                        op=OP.is_equal)
                    at_ps = psm.tile([26, 256], f32, tag="mlp")
                    for ic in range(8):
                        icn = min(128, 1000 - ic * 128)
                        nc.tensor.matmul(
                            out=at_ps[:],
                            lhsT=oh[:icn, ic * 26:(ic + 1) * 26],
                            rhs=w2_sb[:icn, ic * 256:(ic + 1) * 256],
                            start=(ic == 0), stop=(ic == 7))
                    at_sb = pool.tile([26, 256], f16, tag="at_sb")
                    nc.scalar.activation(out=at_sb[:], in_=at_ps[:],
                                         func=AF.Copy)
                    cv_ps = psm.tile([121, 32], f32, tag="mlp")
                    for k in range(8):
                        nc.tensor.matmul(out=cv_ps[:],
                                         lhsT=emb_sb[:, k:k + 121],
                                         rhs=at_sb[:, k * 32:(k + 1) * 32],
                                         start=(k == 0), stop=(k == 7))
                    nc.scalar.activation(out=cvT_sb[:121, b * 32:(b + 1) * 32],
                                         in_=cv_ps[:], func=AF.Copy)

            if "conv" in ablate:
                nc.gpsimd.memset(cvT_sb[:], 0.0)

            # precompute phase-D selection masks during the AG1 window
            wsel_all = cpool.tile([128, T * K * 128], f8, tag="wsel_all")
            for t in range(T):
                rows = slice(t * 128, (t + 1) * 128)
                df = spool.tile([128, K], f32, tag="df")
                nc.sync.dma_start(out=df[:], in_=dstf[rows, :])
                nv = spool.tile([128, K], f32, tag="nv")
                nc.sync.dma_start(out=nv[:], in_=normv[rows, :])
                for c in range(K):
                    nc.vector.tensor_scalar(
                        out=wsel_all[:, (t * K + c) * 128:
                                     (t * K + c + 1) * 128],
                        in0=iota_sb[:], scalar1=df[:, c:c + 1],
                        scalar2=nv[:, c:c + 1],
                        op0=OP.is_equal, op1=OP.mult)

            # ================= Phase B: GAT edge aggregation =============
            # software-pipelined: loads for tile t+1 issue before compute(t)
            MDV = (KT + 1) // 2          # mx chunks on DVE; rest on Pool

            def b_loads(t):
                rows = slice(t * 128, (t + 1) * 128)
                sc = spool.tile([128, K], i32, tag="sc")
                nc.sync.dma_start(out=sc[:], in_=srcs[rows, :])
                df = spool.tile([128, K], f32, tag="df")
                nc.sync.dma_start(out=df[:], in_=dstf[rows, :])
                ade = pool3.tile([128, K * H], f16, tag="ade")
                nc.sync.dma_start(out=ade[:], in_=adste_d[rows, :])
                Gx = pool3.tile([128, KT * XW], f8, tag="G")
                for c in range(K):
                    nc.gpsimd.indirect_dma_start(
                        out=Gx[:, c * XW:(c + 1) * XW], out_offset=None,
                        in_=xtab[:],
                        in_offset=bass.IndirectOffsetOnAxis(
                            ap=sc[:, c:c + 1], axis=0),
                    )
                nc.sync.dma_start(out=Gx[:, K * XW:KT * XW],
                                  in_=xtab_d[rows, :])
                sel = pool3.tile([128, K * 128], f16, tag="sel")
                for c in range(K):
                    nc.vector.tensor_scalar(
                        out=sel[:, c * 128:(c + 1) * 128],
                        in0=iota_sb[:], scalar1=df[:, c:c + 1], scalar2=None,
                        op0=OP.is_equal)
                return rows, Gx, sel, ade

            def b_compute(t, st):
                rows, Gx, sel, ade = st
                al = pool.tile([128, KT * H], f32, tag="al")
                nc.vector.tensor_tensor(
                    out=al[:, :K * H],
                    in0=Gx[:].rearrange("p (k w) -> p k w", w=XW)[:, :K,
                                                                  XF:XW],
                    in1=ade[:].rearrange("p (k h) -> p k h", h=H),
                    op=OP.add)
                nc.vector.tensor_tensor(
                    out=al[:, K * H:KT * H],
                    in0=Gx[:, K * XW + XF:K * XW + XW],
                    in1=adst_sb[:, t * H:(t + 1) * H],
                    op=OP.add)
                al2 = pool.tile([128, KT * H], f32, tag="al2")
                nc.vector.scalar_tensor_tensor(
                    out=al2[:], in0=al[:], scalar=0.2, in1=al[:],
                    op0=OP.mult, op1=OP.max)
                p16 = pool.tile([128, KT * H], f16, tag="p16")
                nc.scalar.activation(out=p16[:], in_=al2[:], func=AF.Exp)
                mx = pool.tile([128, KT * AGW], f16, tag="m")
                nc.vector.tensor_tensor(
                    out=mx[:, :MDV * AGW].rearrange(
                        "p (k h f) -> p k h f", h=H, f=XF),
                    in0=Gx[:].rearrange("p (k w) -> p k w", w=XW)[:, :MDV,
                                                                  :XF]
                         .unsqueeze(2).to_broadcast([128, MDV, H, XF]),
                    in1=p16[:, :MDV * H].rearrange("p (k h) -> p k h", h=H)
                         .unsqueeze(3).to_broadcast([128, MDV, H, XF]),
                    op=OP.mult)
                nc.gpsimd.tensor_tensor(
                    out=mx[:, MDV * AGW:].rearrange(
                        "p (k h f) -> p k h f", h=H, f=XF),
                    in0=Gx[:].rearrange("p (k w) -> p k w", w=XW)[:, MDV:KT,
                                                                  :XF]
                         .unsqueeze(2).to_broadcast([128, KT - MDV, H, XF]),
                    in1=p16[:, MDV * H:].rearrange("p (k h) -> p k h", h=H)
                         .unsqueeze(3).to_broadcast([128, KT - MDV, H, XF]),
                    op=OP.mult)
                aggx_ps = psp.tile([128, AGW], f32, tag="big", name="aggx")
                for c in range(KT):
                    lhsT = (sel[:, c * 128:(c + 1) * 128] if c < K
                            else ident_sb[:])
                    nc.tensor.matmul(out=aggx_ps[:, :512], lhsT=lhsT,
                                     rhs=mx[:, c * AGW: c * AGW + 512],
                                     start=(c == 0), stop=(c == KT - 1))
                    nc.tensor.matmul(out=aggx_ps[:, 512:AGW], lhsT=lhsT,
                                     rhs=mx[:, c * AGW + 512:(c + 1) * AGW],
                                     start=(c == 0), stop=(c == KT - 1))
                rd = spool.tile([128, H], f32, tag="rd")
                nc.vector.reciprocal(
                    out=rd[:],
                    in_=aggx_ps[:].rearrange("p (h f) -> p h f",
                                             f=XF)[:, :, F])
                anorm = pool.tile([128, AGW], f16, tag="anorm")
                nc.vector.tensor_tensor(
                    out=anorm[:].rearrange("p (h f) -> p h f", f=XF),
                    in0=aggx_ps[:].rearrange("p (h f) -> p h f", f=XF),
                    in1=rd[:].unsqueeze(2).to_broadcast([128, H, XF]),
                    op=OP.mult)
                g1_ps = psp.tile([128, HID], f32, tag="big", name="g1p")
                for h in range(H):
                    ht_ps = pst.tile([128, 128], f16, tag="tr")
                    nc.tensor.transpose(out=ht_ps[:XF, :],
                                        in_=anorm[:, h * XF:(h + 1) * XF],
                                        identity=ident_sb[:])
                    hT = pool.tile([128, 128], f16, tag=f"hT{h % 4}")
                    if h % 2 == 0:
                        nc.vector.tensor_copy(out=hT[:XF, :],
                                              in_=ht_ps[:XF, :])
                    else:
                        nc.scalar.activation(out=hT[:XF, :],
                                             in_=ht_ps[:XF, :], func=AF.Copy)
                    for (hh, a, b) in [s for s in HSEG if s[0] == h]:
                        nc.tensor.matmul(out=g1_ps[:, a:b], lhsT=hT[:XF, :],
                                         rhs=whead_sb[:XF, a:b],
                                         start=True, stop=True)
                g1t = pool.tile([128, HID], f8, tag="g1t")
                nc.scalar.activation(out=g1t[:], in_=g1_ps[:], func=AF.Relu)
                nc.sync.dma_start(out=g1_sl[rows, :], in_=g1t[:])

            st = b_loads(0)
            for t in range(T):
                nxt = b_loads(t + 1) if t + 1 < T else None
                b_compute(t, st)
                st = nxt

            if "coll" not in ablate:
                nc.gpsimd.collective_compute(
                    "AllGather", OP.bypass,
                    replica_groups=[list(range(n_cores))],
                    ins=[g1_sl[:]], outs=[g1tab[:]],
                )
            else:
                nc.gpsimd.dma_start(out=g1tab[:NS, :], in_=g1_sl[:])

            # ---- protein branch conv — overlaps AllGather 2 ----
            if "conv" not in ablate:
                conv_block(0, BL)

            # ============ Phase D: GCN aggregation + z + pooling =========
            gmpT_sb = []
            gapT32_sb = []
            gapT_sb = []
            for kk in range(NKK):
                gmpT_sb.append(cpool.tile([128, BL], f16, tag=f"gmpT{kk}",
                                          name=f"gmpT{kk}"))
                gapT32_sb.append(cpool.tile([128, BL], f32, tag=f"gapT32{kk}",
                                            name=f"gapT32{kk}"))
                gapT_sb.append(cpool.tile([128, BL], f16, tag=f"gapT{kk}",
                                          name=f"gapT{kk}"))

            def d_loads(t):
                rows = slice(t * 128, (t + 1) * 128)
                sc = spool.tile([128, K], i32, tag="sc")
                nc.sync.dma_start(out=sc[:], in_=srcs[rows, :])
                G2 = pool3.tile([128, K * HID], f8, tag="G2")
                for c in range(K):
                    nc.gpsimd.indirect_dma_start(
                        out=G2[:, c * HID:(c + 1) * HID], out_offset=None,
                        in_=g1tab[:],
                        in_offset=bass.IndirectOffsetOnAxis(
                            ap=sc[:, c:c + 1], axis=0),
                    )
                g1loc = pool3.tile([128, HID], f8, tag="g1loc")
                nc.sync.dma_start(out=g1loc[:], in_=g1_sl[rows, :])
                return rows, G2, g1loc

            def d_compute(t, st):
                rows, G2, g1loc = st
                dg = pool.tile([128, 128], f8, tag="dg")
                nc.vector.tensor_scalar(out=dg[:], in0=ident_sb[:],
                                        scalar1=nself_sb[:, t:t + 1],
                                        scalar2=None, op0=OP.mult)
                agg_ps = psp.tile([128, HID], f32, tag="big", name="agg")
                for c in range(K + 1):
                    if c < K:
                        wlhs = wsel_all[:, (t * K + c) * 128:
                                        (t * K + c + 1) * 128]
                        rhs_a = G2[:, c * HID: c * HID + 512]
                        rhs_b = G2[:, c * HID + 512:(c + 1) * HID]
                    else:
                        wlhs = dg[:]
                        rhs_a = g1loc[:, :512]
                        rhs_b = g1loc[:, 512:]
                    nc.tensor.matmul(out=agg_ps[:, :512], lhsT=wlhs,
                                     rhs=rhs_a,
                                     start=(c == 0), stop=(c == K))
                    nc.tensor.matmul(out=agg_ps[:, 512:], lhsT=wlhs,
                                     rhs=rhs_b,
                                     start=(c == 0), stop=(c == K))
                s16 = pool.tile([128, HID + 4], f16, tag="s16")
                nc.scalar.activation(out=s16[:, :HID], in_=agg_ps[:],
                                     func=AF.Copy)
                nc.vector.memset(s16[:, HID:HID + 1], 1.0)
                nc.vector.memset(s16[:, HID + 1:HID + 4], 0.0)
                z_ps = psp.tile([128, HID], f32, tag="big", name="z")
                for kk, (ks, kn) in enumerate(FCH):
                    cw = min(16, HID + 4 - ks) if kn < 128 else 128
                    sT_ps = pst.tile([128, 128], f16, tag="tr")
                    nc.tensor.transpose(out=sT_ps[:cw, :],
                                        in_=s16[:, ks:ks + cw],
                                        identity=ident_sb[:])
                    sT = pool.tile([128, 128], f16, tag=f"sT{kk % 4}")
                    nc.scalar.activation(out=sT[:kn, :], in_=sT_ps[:kn, :],
                                         func=AF.Copy)
                    nc.tensor.matmul(
                        out=z_ps[:, :512], lhsT=sT[:kn, :],
                        rhs=gcnw_sb[:kn, kk * HID:kk * HID + 512],
                        start=(kk == 0), stop=(kk == NKK - 1))
                    nc.tensor.matmul(
                        out=z_ps[:, 512:], lhsT=sT[:kn, :],
                        rhs=gcnw_sb[:kn, kk * HID + 512:(kk + 1) * HID],
                        start=(kk == 0), stop=(kk == NKK - 1))
                g2b = pool.tile([128, HID], f16, tag="g2b")
                nc.scalar.activation(out=g2b[:], in_=z_ps[:], func=AF.Relu)
                for kk, (ks, kn) in enumerate(PCH):
                    tp_ps = pst.tile([128, 128], f16, tag="tr")
                    nc.tensor.transpose(out=tp_ps[:kn, :],
                                        in_=g2b[:, ks:ks + kn],
                                        identity=ident_sb[:])
                    nc.vector.reduce_max(
                        out=gmpT_sb[kk][:kn, ng * t:ng * (t + 1)],
                        in_=tp_ps[:kn, :].rearrange("p (g n) -> p g n", n=GN),
                        axis=AX.X)
                    nc.vector.reduce_sum(
                        out=gapT32_sb[kk][:kn, ng * t:ng * (t + 1)],
                        in_=tp_ps[:kn, :].rearrange("p (g n) -> p g n", n=GN),
                        axis=AX.X)

            st = d_loads(0)
            for t in range(T):
                nxt = d_loads(t + 1) if t + 1 < T else None
                d_compute(t, st)
                st = nxt

            for kk, (ks, kn) in enumerate(PCH):
                nc.vector.tensor_copy(out=gapT_sb[kk][:kn, :],
                                      in_=gapT32_sb[kk][:kn, :])

            # ================= Phase E: MLPs =================
            y1_sb = cpool.tile([128, 12 * BL], f16, tag="y1")
            yw_ps = psp.tile([128, 12 * BL], f32, tag="big", name="yw")
            for mi, (ms, mn) in enumerate(tiles(1500)):
                for kk, (ks, kn) in enumerate(PCH):
                    nc.tensor.matmul(
                        out=yw_ps[:mn, mi * BL:(mi + 1) * BL],
                        lhsT=fcg1w_sb[:kn, kk * 1500 + ms:kk * 1500 + ms + mn],
                        rhs=gmpT_sb[kk][:kn, :],
                        start=(kk == 0), stop=False)
                for kk, (ks, kn) in enumerate(PCH):
                    nc.tensor.matmul(
                        out=yw_ps[:mn, mi * BL:(mi + 1) * BL],
                        lhsT=fcg1w_sb[:kn,
                                      (NKK + kk) * 1500 + ms:
                                      (NKK + kk) * 1500 + ms + mn],
                        rhs=gapT_sb[kk][:kn, :],
                        start=False, stop=(kk == NKK - 1))
            for mi, (ms, mn) in enumerate(tiles(1500)):
                nc.scalar.activation(out=y1_sb[:mn, mi * BL:(mi + 1) * BL],
                                     in_=yw_ps[:mn, mi * BL:(mi + 1) * BL],
                                     func=AF.Relu,
                                     bias=fcg1b_sb[:mn, mi:mi + 1])

            xc0 = cpool.tile([128, BL], f16, tag="xc0")
            y2_ps = psm.tile([128, BL], f32, tag="mlp")
            kt2 = tiles(1500)
            for kk, (ks, kn) in enumerate(kt2):
                nc.tensor.matmul(out=y2_ps[:],
                                 lhsT=fcg2w_sb[:kn, kk * 128:(kk + 1) * 128],
                                 rhs=y1_sb[:kn, kk * BL:(kk + 1) * BL],
                                 start=(kk == 0), stop=(kk == len(kt2) - 1))
            nc.vector.tensor_scalar(out=xc0[:], in0=y2_ps[:],
                                    scalar1=fcg2b_sb[:, 0:1], scalar2=None,
                                    op0=OP.add)

            xc1 = cpool.tile([128, BL], f16, tag="xc1")
            xt_ps = psm.tile([128, BL], f32, tag="mlp")
            for o in range(32):
                nc.tensor.matmul(
                    out=xt_ps[:],
                    lhsT=fcxtw_sb[:121, o * 128:(o + 1) * 128],
                    rhs=cvT_sb[:121, :].rearrange("p (b o) -> p b o",
                                                  o=32)[:, :, o],
                    start=(o == 0), stop=(o == 31))
            nc.vector.tensor_scalar(out=xc1[:], in0=xt_ps[:],
                                    scalar1=fcxtb_sb[:, 0:1], scalar2=None,
                                    op0=OP.add)

            # ---- head ----
            y3_sb = cpool.tile([128, 8 * BL], f16, tag="y3")
            y3_ps = psp.tile([128, 8 * BL], f32, tag="big", name="y3p")
            for mi in range(8):
                for kk in range(2):
                    rhs = xc0 if kk == 0 else xc1
                    nc.tensor.matmul(
                        out=y3_ps[:, mi * BL:(mi + 1) * BL],
                        lhsT=fc1w_sb[:, kk * 1024 + mi * 128:
                                     kk * 1024 + (mi + 1) * 128],
                        rhs=rhs[:], start=(kk == 0), stop=(kk == 1))
            for mi in range(8):
                nc.scalar.activation(out=y3_sb[:, mi * BL:(mi + 1) * BL],
                                     in_=y3_ps[:, mi * BL:(mi + 1) * BL],
                                     func=AF.Relu,
                                     bias=fc1b_sb[:, mi:mi + 1])
            y4_sb = cpool.tile([128, 4 * BL], f16, tag="y4")
            y4_ps = psm.tile([128, 4 * BL], f32, tag="mlp")
            for mi in range(4):
                for kk in range(8):
                    nc.tensor.matmul(
                        out=y4_ps[:, mi * BL:(mi + 1) * BL],
                        lhsT=fc2w_sb[:, kk * 512 + mi * 128:
                                     kk * 512 + (mi + 1) * 128],
                        rhs=y3_sb[:, kk * BL:(kk + 1) * BL],
                        start=(kk == 0), stop=(kk == 7))
            for mi in range(4):
                nc.scalar.activation(out=y4_sb[:, mi * BL:(mi + 1) * BL],
                                     in_=y4_ps[:, mi * BL:(mi + 1) * BL],
                                     func=AF.Relu,
                                     bias=fc2b_sb[:, mi:mi + 1])
            o_ps = psm.tile([1, BL], f32, tag="mlp")
            for kk in range(4):
                nc.tensor.matmul(out=o_ps[:], lhsT=outw_sb[:, kk:kk + 1],
                                 rhs=y4_sb[:, kk * BL:(kk + 1) * BL],
                                 start=(kk == 0), stop=(kk == 3))
            o_sb = cpool.tile([1, BL], f32, tag="o_sb")
            nc.vector.tensor_scalar(out=o_sb[:], in0=o_ps[:],
                                    scalar1=outb_sb[:, 0:1], scalar2=None,
                                    op0=OP.add)
            nc.sync.dma_start(out=outp[:], in_=o_sb[:])

    nc.finalize()
    return nc


def run(inp, n_cores=8, trace=False):
    from concourse.bass_utils import run_bass_kernel_spmd
    in_maps, cfg = host_prep(inp, n_cores)
    nc = build(cfg)
    res = run_bass_kernel_spmd(
        nc, in_maps, list(range(n_cores)), trace=trace,
        trace_cores=list(range(n_cores)) if trace else None)
    out = np.concatenate(
        [res.results[c]["outp"].reshape(-1, 1) for c in range(n_cores)], 0)
    return out, res


_CACHED = {}


def kernel(**inputs):
    """Full-input entry point: shards across 8 NeuronCores internally."""
    n_cores = 8
    in_maps, cfg = host_prep(inputs, n_cores)
    key = (cfg["N"], cfg["T"], cfg["BL"], cfg["K"], cfg["GN"])
    nc = _CACHED.get(key)
    if nc is None:
        nc = build(cfg)
        _CACHED[key] = nc
    from concourse.bass_utils import run_bass_kernel_spmd
    res = run_bass_kernel_spmd(nc, in_maps, list(range(n_cores)))
    out = np.concatenate(
        [res.results[c]["outp"].reshape(-1, 1) for c in range(n_cores)], 0)
    return out.astype(np.float32)
